# revision 28
# baseline (speedup 1.0000x reference)
"""Trainium2 Bass kernel for nn_CAKernel_47459388621075.

10 steps of x = clip(x + 0.1*relu(conv5x5_circular(x, W)), 0, 1) on
x:(16,3,1024,1024) f32, W:(3,3,5,5) f32.

Sharding: batch-parallel over 8 NeuronCores (2 images/core) — the circular
conv is per-image, so no cross-core communication is needed.

Per-core kernel: HBM holds the state as 32 packed row-block records per
image, rec[img][b] = [96, 1024] (3 channel bands x 32 rows, rows
O0..O0+31). Each step, each block loads its window tile [108, 1028]:
partitions 0..95 from rec[b] (one large 2D DMA via the SWDGE path, which
spreads packets across all 16 SDMA engines), partitions 96..101 = rows
O0+32,33 from rec[b+1], partitions 102..107 = rows O0-2,O0-1 from rec[b-1]
(two small HWDGE DMAs), plus circular column halos via two DVE copies.

The 5x5x3x3 conv runs as 5 PSUM-accumulated float32r matmuls per 512-col
group (one per kernel column dx); the banded stationary matrix [108, 96]
encodes all 3 input channels x 5 row taps -> out m = 32*co + r. The band
matrices are precomputed host-side from W. Output partitions align with
window partitions, so relu/scale (ACT), add + clip (DVE) are lane-local,
and the store is one full-tile 2D DMA back to rec'[b].
"""
import sys

sys.path.insert(0, "/opt/trn_rl_repo")

import numpy as np

N_CORES = 8
B = 32          # output rows per block
KP = 108        # window partitions: 96 packed + 12 halo
MP = 96         # psum partitions: m = 32*co + r
CG = 512        # matmul column group (PSUM bank)


def make_lhsT(W: np.ndarray) -> np.ndarray:
    """lhsT[p, dx, m]: window partition p -> out m = 32*co + r.

    p = 32*ci + v for in-row offset v in [0,32); p = 96 + 2*ci + (v-32) for
    v in {32,33}; p = 102 + 2*ci + (v+2) for v in {-2,-1}.
    """
    assert W.shape == (3, 3, 5, 5)
    lhsT = np.zeros((KP, 5, MP), dtype=np.float32)
    for r in range(B):
        for dy in range(5):
            v = r + dy - 2
            for ci in range(3):
                if 0 <= v < B:
                    p = 32 * ci + v
                elif v >= B:
                    p = 96 + 2 * ci + (v - B)
                else:
                    p = 102 + 2 * ci + (v + 2)
                for dx in range(5):
                    for co in range(3):
                        lhsT[p, dx, 32 * co + r] = W[co, ci, dy, dx]
    return lhsT.reshape(KP, 5 * MP)


def build_body(tc, x_ap, lhsT_ap, y_ap, n_img, H, Wc, steps):
    """Emit the Tile program. x_ap,(n_img,3,H,Wc) in; y_ap same shape out."""
    from contextlib import ExitStack

    import concourse.bass as bass
    from concourse import mybir

    nc = tc.nc
    f32 = mybir.dt.float32
    f32r = mybir.dt.float32r
    Relu = mybir.ActivationFunctionType.Relu

    n_blocks = H // B
    assert H % B == 0 and Wc % 256 == 0
    n_cg = max(1, Wc // CG)
    cg = Wc // n_cg
    WF = Wc + 4  # window free width (cols -2..Wc+1)

    # packed row-block records, double-buffered across steps
    wb = [
        nc.dram_tensor(f"wbuf{i}", (n_img, n_blocks, MP, WF), f32, kind="Internal").ap()
        for i in range(2)
    ]

    ctx = ExitStack()
    const_pool = ctx.enter_context(tc.tile_pool(name="const", bufs=1))
    win_pool = ctx.enter_context(tc.tile_pool(name="win", bufs=14))
    act_pool = ctx.enter_context(tc.tile_pool(name="act", bufs=8))
    out_pool = ctx.enter_context(tc.tile_pool(name="out", bufs=8))
    psum_pool = ctx.enter_context(tc.tile_pool(name="psum", bufs=8, space="PSUM"))

    lhsT_t = const_pool.tile([KP, 5 * MP], f32r)
    nc.sync.dma_start(lhsT_t[:], lhsT_ap[:, :].bitcast(f32r))

    seq = [(s, img, b) for s in range(steps) for img in range(n_img) for b in range(n_blocks)]
    # load prefetch distance (blocks); must stay behind the producing stores
    # of the previous step (n_img*n_blocks - 2) and below win bufs
    P = max(1, min(12, n_img * n_blocks - 2))

    wins = {}

    def emit_load(i):
        s, img, b = seq[i]
        first = s == 0
        O0 = b * B
        rb = wb[s % 2]
        win = win_pool.tile([KP, WF], f32r, tag="win")
        if first:
            for ci in range(3):
                nc.gpsimd.dma_start(
                    win[32 * ci : 32 * ci + B, 2 : Wc + 2],
                    x_ap[img, ci, O0 : O0 + B, :].bitcast(f32r),
                )
            hn = (O0 + B) % H
            hp = (O0 - 2) % H
            nc.sync.dma_start(
                win[96:102, 2 : Wc + 2], x_ap[img, :, hn : hn + 2, :].bitcast(f32r)
            )
            nc.sync.dma_start(
                win[102:108, 2 : Wc + 2], x_ap[img, :, hp : hp + 2, :].bitcast(f32r)
            )
            # circular column halos (records store them thereafter)
            nc.vector.tensor_copy(win[:, 0:2], win[:, Wc : Wc + 2])
            nc.vector.tensor_copy(win[:, Wc + 2 : Wc + 4], win[:, 2:4])
        else:
            nc.gpsimd.dma_start(win[0:MP, :], rb[img, b].bitcast(f32r))
            rbn = rb[img, (b + 1) % n_blocks].rearrange("(c k) f -> c k f", c=3)
            nc.sync.dma_start(win[96:102, :], rbn[:, 0:2, :].bitcast(f32r))
            if b > 0:
                rbp = rb[img, b - 1].rearrange("(c k) f -> c k f", c=3)
                nc.scalar.dma_start(win[102:108, :], rbp[:, B - 2 : B, :].bitcast(f32r))
            # b == 0 reads the wrap record written at the very end of the
            # previous step — emitted at compute time instead (causality)
        wins[i] = win

    for i in range(min(P, len(seq))):
        emit_load(i)

    DLAG = max(0, min(5, n_img * n_blocks - P - 2))
    pending = []

    for i, (s, img, b) in enumerate(seq):
        last = s == steps - 1
        O0 = b * B
        wbw = wb[(s + 1) % 2]
        win = wins.pop(i)
        if s > 0 and b == 0:
            # scalar-engine queue: its wait on the wrap store doesn't block
            # the SP halo-prefetch stream
            rbp = wb[s % 2][img, n_blocks - 1].rearrange("(c k) f -> c k f", c=3)
            nc.scalar.dma_start(win[102:108, :], rbp[:, B - 2 : B, :].bitcast(f32r))
        if i + P < len(seq):
            emit_load(i + P)

        xn = out_pool.tile([MP, WF], f32)
        for g in range(n_cg):
            psum = psum_pool.tile([MP, cg], f32)
            for dx in range(5):
                nc.tensor.matmul(
                    psum[:],
                    lhsT_t[:, MP * dx : MP * (dx + 1)],
                    win[0:KP, g * cg + dx : g * cg + dx + cg],
                    start=(dx == 0),
                    stop=(dx == 4),
                )
            t = act_pool.tile([MP, cg], f32)
            nc.scalar.activation(t[:], psum[:], Relu, scale=0.1)
            nc.vector.tensor_add(
                xn[:, g * cg + 2 : (g + 1) * cg + 2],
                t[:],
                win[0:MP, g * cg + 2 : (g + 1) * cg + 2].bitcast(f32),
            )
        nc.vector.tensor_scalar_min(xn[:, 2 : Wc + 2], xn[:, 2 : Wc + 2], 1.0)
        # column halos of the outgoing record
        nc.vector.tensor_copy(xn[:, 0:2], xn[:, Wc : Wc + 2])
        nc.vector.tensor_copy(xn[:, Wc + 2 : Wc + 4], xn[:, 2:4])

        # lag stores a couple of blocks so their PL dispatch never waits on
        # in-flight compute (keeps the load prefetch stream moving); flush at
        # step boundaries to preserve next-step wrap causality
        pending.append((img, b, last, O0, xn))
        at_step_end = (i + 1 == len(seq)) or seq[i + 1][0] != s
        while pending and (len(pending) > DLAG or at_step_end):
            pimg, pb, plast, pO0, pxn = pending.pop(0)
            if plast:
                nc.gpsimd.dma_start(
                    y_ap[pimg, :, pO0 : pO0 + B, :], pxn[:, 2 : Wc + 2]
                )
            else:
                nc.gpsimd.dma_start(wbw[pimg, pb], pxn[:])

    ctx.close()


_PROGRAM_CACHE = {}


def _build_program(n_img, H, Wc, steps):
    key = (n_img, H, Wc, steps)
    if key in _PROGRAM_CACHE:
        return _PROGRAM_CACHE[key]
    import concourse.tile as tile
    from concourse import bacc, mybir

    nc = bacc.Bacc(
        "TRN2",
        target_bir_lowering=False,
        debug=False,
        enable_asserts=False,
        num_devices=N_CORES,
    )
    f32 = mybir.dt.float32
    x_ap = nc.dram_tensor("x", (n_img, 3, H, Wc), f32, kind="ExternalInput").ap()
    lhsT_ap = nc.dram_tensor("lhsT", (KP, 5 * MP), f32, kind="ExternalInput").ap()
    y_ap = nc.dram_tensor("y", (n_img, 3, H, Wc), f32, kind="ExternalOutput").ap()
    with tile.TileContext(nc) as tc:
        build_body(tc, x_ap, lhsT_ap, y_ap, n_img, H, Wc, steps)
    nc.compile()
    _PROGRAM_CACHE[key] = nc
    return nc


def kernel(x: np.ndarray, W: np.ndarray, steps) -> np.ndarray:
    from concourse.bass_utils import run_bass_kernel_spmd

    x = np.ascontiguousarray(np.asarray(x), dtype=np.float32)
    W = np.asarray(W, dtype=np.float32)
    steps = int(steps)
    n, c, H, Wc = x.shape
    assert c == 3 and n % N_CORES == 0
    per = n // N_CORES

    nc = _build_program(per, H, Wc, steps)
    lhsT = make_lhsT(W)
    in_maps = [
        {"x": x[i * per : (i + 1) * per], "lhsT": lhsT} for i in range(N_CORES)
    ]
    res = run_bass_kernel_spmd(nc, in_maps, core_ids=list(range(N_CORES)))
    out = np.concatenate([res.results[i]["y"] for i in range(N_CORES)], axis=0)
    return out.astype(np.float32)
